# revision 15
# baseline (speedup 1.0000x reference)
"""DETR loss kernel for 8 trn2 cores.

Device (per core, 8 images): transposed-logit exp + one-hot gather matmuls
(class cost + sumexp), PE-broadcast matmuls emitting bbox-diff / interval
planes, DVE/ACT pipeline for the L1+GIoU cost blocks.
Host: shard prep (transpose, one-hot, SEL coefficient matrices), Hungarian
matching per image (reference does this on host too), final scalar losses
from matched pairs (1600 gathers -> tiny reductions).
"""

import os
from contextlib import ExitStack

import numpy as np

import concourse.bass as bass
import concourse.bacc as bacc
import concourse.mybir as mybir
import concourse.tile as tile
from concourse.bass_utils import run_bass_kernel_spmd

B, Q, T, NCLS = 64, 300, 25, 92  # NCLS = NUM_CLASSES + 1
NCORES = 8
IPC = B // NCORES  # images per core
NA = 28  # aext cols per image: 25 one-hot labels, 1 ones (sumexp), 2 pad
AF = mybir.ActivationFunctionType
ALU = mybir.AluOpType
AX = mybir.AxisListType
F32 = mybir.dt.float32
F32R = mybir.dt.float32  # f32r wedges the exec unit on this HW

# sel column layout (free dim of the box matmuls)
# BD: 4*(25*i+t)+c, c in {dcx,dcy,dw,dh}          cols    0..799
# SA: 800+25*i+t   (area1q + area2t)              cols  800..999
# DPX/DMX/DPY/DMY: d+aq / aq-d planes             cols 1000..1799
# WSX/WSY: wq+wt / hq+ht planes                   cols 1800..2199
NSEL = 2200
NTQ = IPC * T  # 200 (i,t) pairs


def build_program():
    nc = bacc.Bacc("TRN2", target_bir_lowering=False, debug=False)
    # lmeta = [ xt (2400) | aext (224) ] on 92 partitions
    lmeta_d = nc.dram_tensor("lmeta", [NCLS, IPC * Q + IPC * NA], F32R,
                             kind="ExternalInput")
    # bmeta = [ boxext (300) | sel (2200) | tpl-row (400, row 0) ] on 41 parts
    bmeta_d = nc.dram_tensor("bmeta", [41, Q + NSEL + 2 * NTQ], F32R,
                             kind="ExternalInput")
    g_out = nc.dram_tensor("g_out", [NA, IPC * Q], F32, kind="ExternalOutput")
    cbg_out = nc.dram_tensor("cbg_out", [Q, NTQ], F32, kind="ExternalOutput")

    with tile.TileContext(nc) as tc, ExitStack() as ctx:
        sb = ctx.enter_context(tc.tile_pool(name="sb", bufs=1))
        wk = ctx.enter_context(tc.tile_pool(name="wk", bufs=2))
        ps = ctx.enter_context(tc.tile_pool(name="ps", bufs=1, space="PSUM"))
        psg = ctx.enter_context(tc.tile_pool(name="psg", bufs=2, space="PSUM"))

        lm = sb.tile([NCLS, IPC * Q + IPC * NA], F32R, tag="lmeta")
        nc.sync.dma_start(out=lm[:], in_=lmeta_d[:, :])
        xt_t = lm[:, 0:IPC * Q]
        aext_t = lm[:, IPC * Q:]
        bm = sb.tile([41, Q + NSEL + 2 * NTQ], F32R, tag="bmeta")
        nc.sync.dma_start(out=bm[:], in_=bmeta_d[:, :])
        bx = bm[:, 0:Q]
        sl = bm[:, Q:Q + NSEL]
        tp = bm[0:1, Q + NSEL:]

        # broadcast at=(wt/2, ht/2) planes to 100 partitions via ones-matmul
        # (bmeta row 0 cols 0..99 is all-ones)
        ones1 = bm[0:1, 0:100]
        psat = ctx.enter_context(tc.tile_pool(name="psat", bufs=1, space="PSUM"))
        at_ps = psat.tile([100, 2 * NTQ], F32, tag="atps")
        nc.tensor.matmul(out=at_ps[:], lhsT=ones1, rhs=tp,
                         start=True, stop=True)
        at = sb.tile([100, 2 * NTQ], F32, tag="at")
        nc.scalar.copy(out=at[:], in_=at_ps[:])

        # exp(logits^T), then per-image one-hot gather matmul:
        # g[j, q] = sum_c aext[c, j] * exp(x[c, q])
        expxt = sb.tile([NCLS, IPC * Q], F32R, tag="expxt")
        nc.scalar.activation(out=expxt[:], in_=xt_t, func=AF.Exp)
        g_sb = sb.tile([NA, IPC * Q], F32, tag="gsb")
        for i in range(IPC):
            g_ps = psg.tile([NA, Q], F32, tag="gps")
            nc.tensor.matmul(
                out=g_ps[:],
                lhsT=aext_t[:, i * NA:(i + 1) * NA],
                rhs=expxt[:, i * Q:(i + 1) * Q],
                start=True, stop=True,
            )
            nc.scalar.copy(out=g_sb[:, i * Q:(i + 1) * Q], in_=g_ps[:])
        nc.sync.dma_start(out=g_out[:, :], in_=g_sb[:])

        def tt(o, a, b, op):
            nc.vector.tensor_tensor(out=o, in0=a, in1=b, op=op)

        for c in range(3):
            lhs = bx[:, c * 100:(c + 1) * 100]
            psA = ps.tile([100, 500], F32, tag="psA")
            nc.tensor.matmul(out=psA[:], lhsT=lhs, rhs=sl[:, 0:500],
                             start=True, stop=True)
            psB = ps.tile([100, 500], F32, tag="psB")
            nc.tensor.matmul(out=psB[:], lhsT=lhs, rhs=sl[:, 500:1000],
                             start=True, stop=True)
            psC = ps.tile([100, 400], F32, tag="psC")
            nc.tensor.matmul(out=psC[:], lhsT=lhs, rhs=sl[:, 1000:1400],
                             start=True, stop=True)
            psD = ps.tile([100, 400], F32, tag="psD")
            nc.tensor.matmul(out=psD[:], lhsT=lhs, rhs=sl[:, 1400:1800],
                             start=True, stop=True)
            psE = ps.tile([100, 400], F32, tag="psE")
            nc.tensor.matmul(out=psE[:], lhsT=lhs, rhs=sl[:, 1800:2200],
                             start=True, stop=True)

            m1x = wk.tile([100, NTQ], F32, tag="m1x")
            m2x = wk.tile([100, NTQ], F32, tag="m2x")
            wx = wk.tile([100, NTQ], F32, tag="wx")
            m1y = wk.tile([100, NTQ], F32, tag="m1y")
            m2y = wk.tile([100, NTQ], F32, tag="m2y")
            wy = wk.tile([100, NTQ], F32, tag="wy")
            ex = wk.tile([100, NTQ], F32, tag="ex")
            ey = wk.tile([100, NTQ], F32, tag="ey")
            wxr = wk.tile([100, NTQ], F32, tag="wxr")
            wyr = wk.tile([100, NTQ], F32, tag="wyr")
            ii = wk.tile([100, NTQ], F32, tag="ii")
            ae = wk.tile([100, NTQ], F32, tag="ae")
            uu = wk.tile([100, NTQ], F32, tag="uu")
            ue = wk.tile([100, NTQ], F32, tag="ue")
            rue = wk.tile([100, NTQ], F32, tag="rue")
            u2 = wk.tile([100, NTQ], F32, tag="u2")
            ie = wk.tile([100, NTQ], F32, tag="ie")
            nm = wk.tile([100, NTQ], F32, tag="nm")
            ss = wk.tile([100, NTQ], F32, tag="ss")
            bb = wk.tile([100, NTQ], F32, tag="bb")
            cbg = wk.tile([100, NTQ], F32, tag="cbg")

            tt(m1x[:], psC[:, 0:NTQ], at[:, 0:NTQ], ALU.min)
            tt(m2x[:], psC[:, NTQ:2 * NTQ], at[:, 0:NTQ], ALU.min)
            tt(wx[:], m1x[:], m2x[:], ALU.add)
            tt(m1y[:], psD[:, 0:NTQ], at[:, NTQ:2 * NTQ], ALU.min)
            tt(m2y[:], psD[:, NTQ:2 * NTQ], at[:, NTQ:2 * NTQ], ALU.min)
            tt(wy[:], m1y[:], m2y[:], ALU.add)
            tt(ex[:], psE[:, 0:NTQ], wx[:], ALU.subtract)
            tt(ey[:], psE[:, NTQ:2 * NTQ], wy[:], ALU.subtract)
            nc.scalar.activation(out=wxr[:], in_=wx[:], func=AF.Relu)
            nc.scalar.activation(out=wyr[:], in_=wy[:], func=AF.Relu)
            tt(ii[:], wxr[:], wyr[:], ALU.mult)
            tt(ae[:], ex[:], ey[:], ALU.mult)
            tt(uu[:], psB[:, 300:500], ii[:], ALU.subtract)
            # s = I/U + U/E = (I*E + U^2) / (U*E)  -- single reciprocal
            tt(ue[:], uu[:], ae[:], ALU.mult)
            nc.vector.reciprocal(out=rue[:], in_=ue[:])
            nc.scalar.activation(out=u2[:], in_=uu[:], func=AF.Square)
            tt(ie[:], ii[:], ae[:], ALU.mult)
            tt(nm[:], ie[:], u2[:], ALU.add)
            tt(ss[:], nm[:], rue[:], ALU.mult)
            nc.vector.tensor_reduce(
                out=bb[:, 0:125], in_=psA[:].rearrange("p (g c) -> p g c", c=4),
                axis=AX.X, op=ALU.add, apply_absolute_value=True)
            nc.vector.tensor_reduce(
                out=bb[:, 125:200], in_=psB[:, 0:300].rearrange("p (g c) -> p g c", c=4),
                axis=AX.X, op=ALU.add, apply_absolute_value=True)
            # cbg = bbox - 0.4*s ; host: C = clsc + 5*cbg + 2
            nc.vector.scalar_tensor_tensor(
                out=cbg[:], in0=ss[:], scalar=-0.4, in1=bb[:],
                op0=ALU.mult, op1=ALU.add)
            nc.sync.dma_start(out=cbg_out[c * 100:(c + 1) * 100, :], in_=cbg[:])
    return nc


def prep_core_inputs(pred_logits, pred_boxes, tgt_labels, tgt_boxes, k):
    s = slice(k * IPC, (k + 1) * IPC)
    pl = np.asarray(pred_logits[s], np.float32)   # [8,300,92]
    pb = np.asarray(pred_boxes[s], np.float32)    # [8,300,4]
    tl = np.asarray(tgt_labels[s], np.int64)      # [8,25]
    tb = np.asarray(tgt_boxes[s], np.float32)     # [8,25,4]

    xt = np.ascontiguousarray(pl.transpose(2, 0, 1).reshape(NCLS, IPC * Q))

    aext = np.zeros((NCLS, IPC * NA), np.float32)
    for i in range(IPC):
        aext[tl[i, :], NA * i + np.arange(T)] = 1.0
        aext[:, NA * i + T] = 1.0

    cx, cy, w, h = pb[..., 0], pb[..., 1], pb[..., 2], pb[..., 3]  # [8,300]
    boxext = np.zeros((41, Q), np.float32)
    for i in range(IPC):
        boxext[1 + 5 * i + 0] = cx[i]
        boxext[1 + 5 * i + 1] = cy[i]
        boxext[1 + 5 * i + 2] = w[i]
        boxext[1 + 5 * i + 3] = h[i]
        boxext[1 + 5 * i + 4] = w[i] * h[i]
    boxext[0] = 1.0

    tcx, tcy, tw, th = tb[..., 0], tb[..., 1], tb[..., 2], tb[..., 3]  # [8,25]
    sel = np.zeros((41, NSEL), np.float32)
    ar = np.arange(T)
    for i in range(IPC):
        for c_, (row, tv) in enumerate(((0, tcx), (1, tcy), (2, tw), (3, th))):
            cols = 4 * (T * i + ar) + c_
            sel[1 + 5 * i + row, cols] = 1.0
            sel[0, cols] = -tv[i]
        cols = 800 + T * i + ar
        sel[1 + 5 * i + 4, cols] = 1.0
        sel[0, cols] = tw[i] * th[i]
        cols = 1000 + T * i + ar  # D+x = dx + aq
        sel[1 + 5 * i + 0, cols] = 1.0
        sel[1 + 5 * i + 2, cols] = 0.5
        sel[0, cols] = -tcx[i]
        cols = 1200 + T * i + ar  # D-x = aq - dx
        sel[1 + 5 * i + 0, cols] = -1.0
        sel[1 + 5 * i + 2, cols] = 0.5
        sel[0, cols] = tcx[i]
        cols = 1400 + T * i + ar
        sel[1 + 5 * i + 1, cols] = 1.0
        sel[1 + 5 * i + 3, cols] = 0.5
        sel[0, cols] = -tcy[i]
        cols = 1600 + T * i + ar
        sel[1 + 5 * i + 1, cols] = -1.0
        sel[1 + 5 * i + 3, cols] = 0.5
        sel[0, cols] = tcy[i]
        cols = 1800 + T * i + ar  # WSx = wq + wt
        sel[1 + 5 * i + 2, cols] = 1.0
        sel[0, cols] = tw[i]
        cols = 2000 + T * i + ar  # WSy = hq + ht
        sel[1 + 5 * i + 3, cols] = 1.0
        sel[0, cols] = th[i]

    tpl = np.concatenate([(tw / 2).reshape(-1), (th / 2).reshape(-1)])
    tpl = np.ascontiguousarray(tpl.reshape(1, 2 * NTQ), np.float32)

    lmeta = np.concatenate([xt, aext], axis=1)
    bmeta = np.zeros((41, Q + NSEL + 2 * NTQ), np.float32)
    bmeta[:, 0:Q] = boxext
    bmeta[:, Q:Q + NSEL] = sel
    bmeta[0, Q + NSEL:] = tpl[0]
    return {"lmeta": lmeta, "bmeta": bmeta}


def _lsa(cost):
    # Jonker-Volgenant, identical to the reference implementation
    cost = np.asarray(cost, dtype=np.float64)
    transposed = cost.shape[0] > cost.shape[1]
    if transposed:
        cost = cost.T
    n, m = cost.shape
    u = np.zeros(n + 1)
    v = np.zeros(m + 1)
    p = np.zeros(m + 1, dtype=np.int64)
    way = np.zeros(m + 1, dtype=np.int64)
    for i in range(1, n + 1):
        p[0] = i
        j0 = 0
        minv = np.full(m + 1, np.inf)
        used = np.zeros(m + 1, dtype=bool)
        while True:
            used[j0] = True
            i0 = p[j0]
            cur = cost[i0 - 1, :] - u[i0] - v[1:]
            free = ~used[1:]
            better = free & (cur < minv[1:])
            minv[1:][better] = cur[better]
            way[1:][better] = j0
            masked = np.where(free, minv[1:], np.inf)
            j1 = int(np.argmin(masked)) + 1
            delta = minv[j1]
            u[p[used]] += delta
            v[used] -= delta
            minv[~used] -= delta
            j0 = j1
            if p[j0] == 0:
                break
        while j0:
            j1 = way[j0]
            p[j0] = p[j1]
            j0 = j1
    rows, cols = [], []
    for j in range(1, m + 1):
        if p[j] != 0:
            rows.append(p[j] - 1)
            cols.append(j - 1)
    rows = np.asarray(rows, dtype=np.int64)
    cols = np.asarray(cols, dtype=np.int64)
    if transposed:
        rows, cols = cols, rows
    order = np.argsort(rows)
    return rows[order], cols[order]


def _xyxy(b):
    cx, cy, w, h = b[..., 0], b[..., 1], b[..., 2], b[..., 3]
    return np.stack([cx - w / 2, cy - h / 2, cx + w / 2, cy + h / 2], -1)


_PROG = None


def kernel(pred_logits, pred_boxes, tgt_labels, tgt_boxes):
    global _PROG
    pred_logits = np.asarray(pred_logits, np.float32)
    pred_boxes = np.asarray(pred_boxes, np.float32)
    tgt_labels_in = np.asarray(tgt_labels)
    tgt_labels = tgt_labels_in.astype(np.int64)
    tgt_boxes = np.asarray(tgt_boxes, np.float32)

    if _PROG is None:
        _PROG = build_program()
        if not _PROG.is_finalized():
            _PROG.finalize()
    nc = _PROG
    in_maps = [
        prep_core_inputs(pred_logits, pred_boxes, tgt_labels, tgt_boxes, k)
        for k in range(NCORES)
    ]
    trace = bool(int(os.environ.get("KERNEL_TRACE", "0")))
    res = run_bass_kernel_spmd(nc, in_maps, list(range(NCORES)), trace=trace)
    kernel.last_results = res
    outs = res.results

    # reassemble per-image cost matrices and logZ
    logZ = np.zeros((B, Q))
    C_all = np.zeros((B, Q, T))
    for k in range(NCORES):
        g = np.asarray(outs[k]["g_out"], np.float64)      # [28, 2400]
        cbg = np.asarray(outs[k]["cbg_out"], np.float64)  # [300, 200]
        for i in range(IPC):
            b = k * IPC + i
            gi = g[:, i * Q:(i + 1) * Q]
            lz = np.log(gi[T])
            logZ[b] = lz
            clsc = lz[None, :] - np.log(np.maximum(gi[0:T], 1e-30))  # [25,300]
            C_all[b] = clsc.T + 5.0 * cbg[:, i * T:(i + 1) * T] + 2.0

    # Hungarian per image (host, like the reference)
    src = np.zeros((B, T), np.int64)
    tgt = np.zeros((B, T), np.int64)
    for b in range(B):
        r, c = _lsa(C_all[b])
        src[b] = r
        tgt[b] = c

    bidx = np.repeat(np.arange(B), T)
    sidx = src.reshape(-1)
    tidx = tgt.reshape(-1)
    num_t = B * T

    pl64 = pred_logits.astype(np.float64)
    pb64 = pred_boxes.astype(np.float64)
    tb64 = tgt_boxes.astype(np.float64)

    nll91 = logZ - pl64[:, :, 91]
    labels_m = tgt_labels[bidx, tidx]
    nll_m = logZ[bidx, sidx] - pl64[bidx, sidx, labels_m]
    ce_num = 0.1 * nll91.sum() + (nll_m - 0.1 * nll91[bidx, sidx]).sum()
    ce_den = 0.1 * (B * Q - num_t) + 1.0 * num_t
    loss_ce = ce_num / ce_den

    sb = pb64[bidx, sidx]
    tb_m = tb64[bidx, tidx]
    loss_bbox = np.abs(sb - tb_m).sum() / num_t

    b1 = _xyxy(sb)
    b2 = _xyxy(tb_m)
    a1 = (b1[:, 2] - b1[:, 0]) * (b1[:, 3] - b1[:, 1])
    a2 = (b2[:, 2] - b2[:, 0]) * (b2[:, 3] - b2[:, 1])
    lt = np.maximum(b1[:, :2], b2[:, :2])
    rb = np.minimum(b1[:, 2:], b2[:, 2:])
    wh = np.clip(rb - lt, 0, None)
    inter = wh[:, 0] * wh[:, 1]
    union = a1 + a2 - inter
    iou = inter / union
    lti = np.minimum(b1[:, :2], b2[:, :2])
    rbi = np.maximum(b1[:, 2:], b2[:, 2:])
    whi = np.clip(rbi - lti, 0, None)
    areai = whi[:, 0] * whi[:, 1]
    giou = iou - (areai - union) / areai
    loss_giou = (1.0 - giou).sum() / num_t

    l_ce = 2.0 * loss_ce
    l_bb = 2.5 * loss_bbox
    l_gi = 2.0 * loss_giou
    return np.array([l_ce, l_bb, l_gi, l_ce + l_bb + l_gi], np.float32)


# revision 18
# speedup vs baseline: 1.3441x; 1.3441x over previous
"""DETR loss kernel for 8 trn2 cores.

Device (per core, 8 images): transposed-logit exp + one-hot gather matmuls
(class cost + sumexp), PE-broadcast matmuls emitting bbox-diff / interval
planes, DVE/ACT pipeline for the L1+GIoU cost blocks.
Host: shard prep (transpose, one-hot, SEL coefficient matrices), Hungarian
matching per image (reference does this on host too), final scalar losses
from matched pairs (1600 gathers -> tiny reductions).
"""

import os
from contextlib import ExitStack

import numpy as np

import concourse.bass as bass
import concourse.bacc as bacc
import concourse.mybir as mybir
import concourse.tile as tile
from concourse.bass_utils import run_bass_kernel_spmd

B, Q, T, NCLS = 64, 300, 25, 92  # NCLS = NUM_CLASSES + 1
NCORES = 8
IPC = B // NCORES  # images per core
NA = 28  # aext cols per image: 25 one-hot labels, 1 ones (sumexp), 2 pad
AF = mybir.ActivationFunctionType
ALU = mybir.AluOpType
AX = mybir.AxisListType
F32 = mybir.dt.float32
BF16 = mybir.dt.bfloat16  # f32r wedges the exec unit; bf16 matmuls are 1 cyc/row

# sel column layout (free dim of the box matmuls)
# BD: 4*(25*i+t)+c, c in {dcx,dcy,dw,dh}          cols    0..799
# SA: 800+25*i+t   (area1q + area2t)              cols  800..999
# DPX/DMX/DPY/DMY: d+aq / aq-d planes             cols 1000..1799
# WSX/WSY: wq+wt / hq+ht planes                   cols 1800..2199
NSEL = 2200
NTQ = IPC * T  # 200 (i,t) pairs


def build_program():
    nc = bacc.Bacc("TRN2", target_bir_lowering=False, debug=False)
    # lmeta = [ xt (2400) | aext (224) ] on 92 partitions
    lmeta_d = nc.dram_tensor("lmeta", [NCLS, IPC * Q + IPC * NA], BF16,
                             kind="ExternalInput")
    # bmeta = [ boxext (300) | sel (2200) | tpl-row (400, row 0) ] on 41 parts
    bmeta_d = nc.dram_tensor("bmeta", [41, Q + NSEL + 2 * NTQ], BF16,
                             kind="ExternalInput")
    g_out = nc.dram_tensor("g_out", [NA, IPC * Q], F32, kind="ExternalOutput")
    cbg_out = nc.dram_tensor("cbg_out", [Q, NTQ], F32, kind="ExternalOutput")

    with tile.TileContext(nc) as tc, ExitStack() as ctx:
        sb = ctx.enter_context(tc.tile_pool(name="sb", bufs=1))
        wk = ctx.enter_context(tc.tile_pool(name="wk", bufs=2))
        ps = ctx.enter_context(tc.tile_pool(name="ps", bufs=1, space="PSUM"))
        psg = ctx.enter_context(tc.tile_pool(name="psg", bufs=2, space="PSUM"))

        lm = sb.tile([NCLS, IPC * Q + IPC * NA], BF16, tag="lmeta")
        nc.sync.dma_start(out=lm[:], in_=lmeta_d[:, :])
        xt_t = lm[:, 0:IPC * Q]
        aext_t = lm[:, IPC * Q:]
        bm = sb.tile([41, Q + NSEL + 2 * NTQ], BF16, tag="bmeta")
        nc.sync.dma_start(out=bm[:], in_=bmeta_d[:, :])
        bx = bm[:, 0:Q]
        sl = bm[:, Q:Q + NSEL]
        tp = bm[0:1, Q + NSEL:]

        # broadcast at=(wt/2, ht/2) planes to 100 partitions via ones-matmul
        # (bmeta row 0 cols 0..99 is all-ones)
        ones1 = bm[0:1, 0:100]
        psat = ctx.enter_context(tc.tile_pool(name="psat", bufs=1, space="PSUM"))
        at_ps = psat.tile([100, 2 * NTQ], F32, tag="atps")
        nc.tensor.matmul(out=at_ps[:], lhsT=ones1, rhs=tp,
                         start=True, stop=True)
        at1 = sb.tile([100, NTQ], F32, tag="at1")
        nc.scalar.copy(out=at1[:], in_=at_ps[:, 0:NTQ])
        at2 = sb.tile([100, NTQ], F32, tag="at2")
        nc.scalar.copy(out=at2[:], in_=at_ps[:, NTQ:2 * NTQ])

        # exp(logits^T), then per-image one-hot gather matmul:
        # g[j, q] = sum_c aext[c, j] * exp(x[c, q])
        expxt = sb.tile([NCLS, IPC * Q], BF16, tag="expxt")
        nc.scalar.activation(out=expxt[:], in_=xt_t, func=AF.Exp)
        g_sb = sb.tile([NA, IPC * Q], F32, tag="gsb")
        for i in range(IPC):
            g_ps = psg.tile([NA, Q], F32, tag="gps")
            nc.tensor.matmul(
                out=g_ps[:],
                lhsT=aext_t[:, i * NA:(i + 1) * NA],
                rhs=expxt[:, i * Q:(i + 1) * Q],
                start=True, stop=True,
            )
            nc.scalar.copy(out=g_sb[:, i * Q:(i + 1) * Q], in_=g_ps[:])
        nc.sync.dma_start(out=g_out[:, :], in_=g_sb[:])

        def tt(o, a, b, op):
            nc.vector.tensor_tensor(out=o, in0=a, in1=b, op=op)

        def gt(o, a, b, op):
            nc.gpsimd.tensor_tensor(out=o, in0=a, in1=b, op=op)

        for c in range(3):
            lhs = bx[:, c * 100:(c + 1) * 100]
            psA = ps.tile([100, 500], F32, tag="psA")
            nc.tensor.matmul(out=psA[:], lhsT=lhs, rhs=sl[:, 0:500],
                             start=True, stop=True)
            psB = ps.tile([100, 500], F32, tag="psB")
            nc.tensor.matmul(out=psB[:], lhsT=lhs, rhs=sl[:, 500:1000],
                             start=True, stop=True)
            psC = ps.tile([100, 400], F32, tag="psC")
            nc.tensor.matmul(out=psC[:], lhsT=lhs, rhs=sl[:, 1000:1400],
                             start=True, stop=True)
            psD = ps.tile([100, 400], F32, tag="psD")
            nc.tensor.matmul(out=psD[:], lhsT=lhs, rhs=sl[:, 1400:1800],
                             start=True, stop=True)
            psE = ps.tile([100, 400], F32, tag="psE")
            nc.tensor.matmul(out=psE[:], lhsT=lhs, rhs=sl[:, 1800:2200],
                             start=True, stop=True)

            m1x = wk.tile([100, NTQ], F32, tag="m1x")
            m2x = wk.tile([100, NTQ], F32, tag="m2x")
            wx = wk.tile([100, NTQ], F32, tag="wx")
            m1y = wk.tile([100, NTQ], F32, tag="m1y")
            m2y = wk.tile([100, NTQ], F32, tag="m2y")
            wy = wk.tile([100, NTQ], F32, tag="wy")
            ex = wk.tile([100, NTQ], F32, tag="ex")
            ey = wk.tile([100, NTQ], F32, tag="ey")
            wxr = wk.tile([100, NTQ], F32, tag="wxr")
            wyr = wk.tile([100, NTQ], F32, tag="wyr")
            ii = wk.tile([100, NTQ], F32, tag="ii")
            ae = wk.tile([100, NTQ], F32, tag="ae")
            uu = wk.tile([100, NTQ], F32, tag="uu")
            ue = wk.tile([100, NTQ], F32, tag="ue")
            rue = wk.tile([100, NTQ], F32, tag="rue")
            u2 = wk.tile([100, NTQ], F32, tag="u2")
            ie = wk.tile([100, NTQ], F32, tag="ie")
            nm = wk.tile([100, NTQ], F32, tag="nm")
            ss = wk.tile([100, NTQ], F32, tag="ss")
            bb = wk.tile([100, NTQ], F32, tag="bb")
            cbg = wk.tile([100, NTQ], F32, tag="cbg")

            tt(m1x[:], psC[:, 0:NTQ], at1[:], ALU.min)
            tt(m2x[:], psC[:, NTQ:2 * NTQ], at1[:], ALU.min)
            tt(wx[:], m1x[:], m2x[:], ALU.add)
            tt(m1y[:], psD[:, 0:NTQ], at2[:], ALU.min)
            tt(m2y[:], psD[:, NTQ:2 * NTQ], at2[:], ALU.min)
            gt(wy[:], m1y[:], m2y[:], ALU.add)
            tt(ex[:], psE[:, 0:NTQ], wx[:], ALU.subtract)
            tt(ey[:], psE[:, NTQ:2 * NTQ], wy[:], ALU.subtract)
            nc.scalar.activation(out=wxr[:], in_=wx[:], func=AF.Relu)
            nc.scalar.activation(out=wyr[:], in_=wy[:], func=AF.Relu)
            tt(ii[:], wxr[:], wyr[:], ALU.mult)
            gt(ae[:], ex[:], ey[:], ALU.mult)
            tt(uu[:], psB[:, 300:500], ii[:], ALU.subtract)
            # s = I/U + U/E = (I*E + U^2) / (U*E)  -- single reciprocal
            gt(ue[:], uu[:], ae[:], ALU.mult)
            nc.vector.reciprocal_approx_fast(out=rue[:], in_=ue[:])
            nc.scalar.activation(out=u2[:], in_=uu[:], func=AF.Square)
            gt(ie[:], ii[:], ae[:], ALU.mult)
            tt(nm[:], ie[:], u2[:], ALU.add)
            tt(ss[:], nm[:], rue[:], ALU.mult)
            nc.vector.tensor_reduce(
                out=bb[:, 0:125], in_=psA[:].rearrange("p (g c) -> p g c", c=4),
                axis=AX.X, op=ALU.add, apply_absolute_value=True)
            nc.vector.tensor_reduce(
                out=bb[:, 125:200], in_=psB[:, 0:300].rearrange("p (g c) -> p g c", c=4),
                axis=AX.X, op=ALU.add, apply_absolute_value=True)
            # cbg = bbox - 0.4*s ; host: C = clsc + 5*cbg + 2
            nc.vector.scalar_tensor_tensor(
                out=cbg[:], in0=ss[:], scalar=-0.4, in1=bb[:],
                op0=ALU.mult, op1=ALU.add)
            nc.sync.dma_start(out=cbg_out[c * 100:(c + 1) * 100, :], in_=cbg[:])
    return nc


def prep_core_inputs(pred_logits, pred_boxes, tgt_labels, tgt_boxes, k):
    s = slice(k * IPC, (k + 1) * IPC)
    pl = np.asarray(pred_logits[s], np.float32)   # [8,300,92]
    pb = np.asarray(pred_boxes[s], np.float32)    # [8,300,4]
    tl = np.asarray(tgt_labels[s], np.int64)      # [8,25]
    tb = np.asarray(tgt_boxes[s], np.float32)     # [8,25,4]

    xt = np.ascontiguousarray(pl.transpose(2, 0, 1).reshape(NCLS, IPC * Q))

    aext = np.zeros((NCLS, IPC * NA), np.float32)
    for i in range(IPC):
        aext[tl[i, :], NA * i + np.arange(T)] = 1.0
        aext[:, NA * i + T] = 1.0

    cx, cy, w, h = pb[..., 0], pb[..., 1], pb[..., 2], pb[..., 3]  # [8,300]
    boxext = np.zeros((41, Q), np.float32)
    for i in range(IPC):
        boxext[1 + 5 * i + 0] = cx[i]
        boxext[1 + 5 * i + 1] = cy[i]
        boxext[1 + 5 * i + 2] = w[i]
        boxext[1 + 5 * i + 3] = h[i]
        boxext[1 + 5 * i + 4] = w[i] * h[i]
    boxext[0] = 1.0

    tcx, tcy, tw, th = tb[..., 0], tb[..., 1], tb[..., 2], tb[..., 3]  # [8,25]
    sel = np.zeros((41, NSEL), np.float32)
    ar = np.arange(T)
    for i in range(IPC):
        for c_, (row, tv) in enumerate(((0, tcx), (1, tcy), (2, tw), (3, th))):
            cols = 4 * (T * i + ar) + c_
            sel[1 + 5 * i + row, cols] = 1.0
            sel[0, cols] = -tv[i]
        cols = 800 + T * i + ar
        sel[1 + 5 * i + 4, cols] = 1.0
        sel[0, cols] = tw[i] * th[i]
        cols = 1000 + T * i + ar  # D+x = dx + aq
        sel[1 + 5 * i + 0, cols] = 1.0
        sel[1 + 5 * i + 2, cols] = 0.5
        sel[0, cols] = -tcx[i]
        cols = 1200 + T * i + ar  # D-x = aq - dx
        sel[1 + 5 * i + 0, cols] = -1.0
        sel[1 + 5 * i + 2, cols] = 0.5
        sel[0, cols] = tcx[i]
        cols = 1400 + T * i + ar
        sel[1 + 5 * i + 1, cols] = 1.0
        sel[1 + 5 * i + 3, cols] = 0.5
        sel[0, cols] = -tcy[i]
        cols = 1600 + T * i + ar
        sel[1 + 5 * i + 1, cols] = -1.0
        sel[1 + 5 * i + 3, cols] = 0.5
        sel[0, cols] = tcy[i]
        cols = 1800 + T * i + ar  # WSx = wq + wt
        sel[1 + 5 * i + 2, cols] = 1.0
        sel[0, cols] = tw[i]
        cols = 2000 + T * i + ar  # WSy = hq + ht
        sel[1 + 5 * i + 3, cols] = 1.0
        sel[0, cols] = th[i]

    tpl = np.concatenate([(tw / 2).reshape(-1), (th / 2).reshape(-1)])
    tpl = np.ascontiguousarray(tpl.reshape(1, 2 * NTQ), np.float32)

    import ml_dtypes
    lmeta = np.concatenate([xt, aext], axis=1).astype(ml_dtypes.bfloat16)
    bmeta = np.zeros((41, Q + NSEL + 2 * NTQ), np.float32)
    bmeta[:, 0:Q] = boxext
    bmeta[:, Q:Q + NSEL] = sel
    bmeta[0, Q + NSEL:] = tpl[0]
    return {"lmeta": lmeta, "bmeta": bmeta.astype(ml_dtypes.bfloat16)}


def _lsa(cost):
    # Jonker-Volgenant, identical to the reference implementation
    cost = np.asarray(cost, dtype=np.float64)
    transposed = cost.shape[0] > cost.shape[1]
    if transposed:
        cost = cost.T
    n, m = cost.shape
    u = np.zeros(n + 1)
    v = np.zeros(m + 1)
    p = np.zeros(m + 1, dtype=np.int64)
    way = np.zeros(m + 1, dtype=np.int64)
    for i in range(1, n + 1):
        p[0] = i
        j0 = 0
        minv = np.full(m + 1, np.inf)
        used = np.zeros(m + 1, dtype=bool)
        while True:
            used[j0] = True
            i0 = p[j0]
            cur = cost[i0 - 1, :] - u[i0] - v[1:]
            free = ~used[1:]
            better = free & (cur < minv[1:])
            minv[1:][better] = cur[better]
            way[1:][better] = j0
            masked = np.where(free, minv[1:], np.inf)
            j1 = int(np.argmin(masked)) + 1
            delta = minv[j1]
            u[p[used]] += delta
            v[used] -= delta
            minv[~used] -= delta
            j0 = j1
            if p[j0] == 0:
                break
        while j0:
            j1 = way[j0]
            p[j0] = p[j1]
            j0 = j1
    rows, cols = [], []
    for j in range(1, m + 1):
        if p[j] != 0:
            rows.append(p[j] - 1)
            cols.append(j - 1)
    rows = np.asarray(rows, dtype=np.int64)
    cols = np.asarray(cols, dtype=np.int64)
    if transposed:
        rows, cols = cols, rows
    order = np.argsort(rows)
    return rows[order], cols[order]


def _xyxy(b):
    cx, cy, w, h = b[..., 0], b[..., 1], b[..., 2], b[..., 3]
    return np.stack([cx - w / 2, cy - h / 2, cx + w / 2, cy + h / 2], -1)


_PROG = None


def kernel(pred_logits, pred_boxes, tgt_labels, tgt_boxes):
    global _PROG
    pred_logits = np.asarray(pred_logits, np.float32)
    pred_boxes = np.asarray(pred_boxes, np.float32)
    tgt_labels_in = np.asarray(tgt_labels)
    tgt_labels = tgt_labels_in.astype(np.int64)
    tgt_boxes = np.asarray(tgt_boxes, np.float32)

    if _PROG is None:
        _PROG = build_program()
        if not _PROG.is_finalized():
            _PROG.finalize()
    nc = _PROG
    in_maps = [
        prep_core_inputs(pred_logits, pred_boxes, tgt_labels, tgt_boxes, k)
        for k in range(NCORES)
    ]
    trace = bool(int(os.environ.get("KERNEL_TRACE", "0")))
    res = run_bass_kernel_spmd(nc, in_maps, list(range(NCORES)), trace=trace)
    kernel.last_results = res
    outs = res.results

    # reassemble per-image cost matrices and logZ
    logZ = np.zeros((B, Q))
    C_all = np.zeros((B, Q, T))
    for k in range(NCORES):
        g = np.asarray(outs[k]["g_out"], np.float64)      # [28, 2400]
        cbg = np.asarray(outs[k]["cbg_out"], np.float64)  # [300, 200]
        for i in range(IPC):
            b = k * IPC + i
            gi = g[:, i * Q:(i + 1) * Q]
            lz = np.log(gi[T])
            logZ[b] = lz
            clsc = lz[None, :] - np.log(np.maximum(gi[0:T], 1e-30))  # [25,300]
            C_all[b] = clsc.T + 5.0 * cbg[:, i * T:(i + 1) * T] + 2.0

    # Hungarian per image (host, like the reference)
    src = np.zeros((B, T), np.int64)
    tgt = np.zeros((B, T), np.int64)
    for b in range(B):
        r, c = _lsa(C_all[b])
        src[b] = r
        tgt[b] = c

    bidx = np.repeat(np.arange(B), T)
    sidx = src.reshape(-1)
    tidx = tgt.reshape(-1)
    num_t = B * T

    pl64 = pred_logits.astype(np.float64)
    pb64 = pred_boxes.astype(np.float64)
    tb64 = tgt_boxes.astype(np.float64)

    nll91 = logZ - pl64[:, :, 91]
    labels_m = tgt_labels[bidx, tidx]
    nll_m = logZ[bidx, sidx] - pl64[bidx, sidx, labels_m]
    ce_num = 0.1 * nll91.sum() + (nll_m - 0.1 * nll91[bidx, sidx]).sum()
    ce_den = 0.1 * (B * Q - num_t) + 1.0 * num_t
    loss_ce = ce_num / ce_den

    sb = pb64[bidx, sidx]
    tb_m = tb64[bidx, tidx]
    loss_bbox = np.abs(sb - tb_m).sum() / num_t

    b1 = _xyxy(sb)
    b2 = _xyxy(tb_m)
    a1 = (b1[:, 2] - b1[:, 0]) * (b1[:, 3] - b1[:, 1])
    a2 = (b2[:, 2] - b2[:, 0]) * (b2[:, 3] - b2[:, 1])
    lt = np.maximum(b1[:, :2], b2[:, :2])
    rb = np.minimum(b1[:, 2:], b2[:, 2:])
    wh = np.clip(rb - lt, 0, None)
    inter = wh[:, 0] * wh[:, 1]
    union = a1 + a2 - inter
    iou = inter / union
    lti = np.minimum(b1[:, :2], b2[:, :2])
    rbi = np.maximum(b1[:, 2:], b2[:, 2:])
    whi = np.clip(rbi - lti, 0, None)
    areai = whi[:, 0] * whi[:, 1]
    giou = iou - (areai - union) / areai
    loss_giou = (1.0 - giou).sum() / num_t

    l_ce = 2.0 * loss_ce
    l_bb = 2.5 * loss_bbox
    l_gi = 2.0 * loss_giou
    return np.array([l_ce, l_bb, l_gi, l_ce + l_bb + l_gi], np.float32)
